# revision 14
# baseline (speedup 1.0000x reference)
"""Distributed MultiHeadAttention (B=2, N=2048, D=2048, H=16, dk=128) on 8 TRN2 cores.

Sharding: head-parallel (2 heads/core, both batches => 4 (b,h) instances/core).
All projections/attention computed in transposed layout ([dim, seq]).
AllToAll reshards attention outputs so out_proj is sequence-parallel
(each core produces 512 rows of the final output).
"""
import numpy as np

import concourse.bacc as bacc
import concourse.mybir as mybir
import concourse.tile as tile
from concourse.bass_utils import run_bass_kernel_spmd
from concourse.masks import make_identity, make_causal_mask

F32 = mybir.dt.float32
F32R = mybir.dt.float32r
BF16 = mybir.dt.bfloat16

NC = 8          # cores
B = 2
N = 2048
D = 2048
H = 16
DK = 128
HPC = H // NC   # heads per core = 2
INST = B * HPC  # (b, h) instances per core = 4
BN = B * N      # 4096 flattened rows
ROWS_PER_CORE = BN // NC  # 512
NT = 16         # 128-row tiles in D (k-tiles)
ITILES = N // 128  # 16 i-tiles per instance
XC = 256        # x chunk width in phase 1
MASK_VAL = -1e30


def _r(ap):
    return ap.bitcast(F32R)


def build_kernel():
    nc = bacc.Bacc("TRN2", num_devices=NC)

    # ---------------- DRAM I/O ----------------
    xT_d = nc.dram_tensor("xT", [D, BN], F32R, kind="ExternalInput")
    wqT_d = nc.dram_tensor("wqT", [D, HPC * DK], F32R, kind="ExternalInput")
    wkT_d = nc.dram_tensor("wkT", [D, HPC * DK], F32R, kind="ExternalInput")
    wvT_d = nc.dram_tensor("wvT", [D, HPC * DK], F32R, kind="ExternalInput")
    woT_d = nc.dram_tensor("woT", [D, D], F32R, kind="ExternalInput")
    cosT_d = nc.dram_tensor("cosT", [128, N], F32, kind="ExternalInput")
    sinT_d = nc.dram_tensor("sinT", [128, N], F32, kind="ExternalInput")

    y_d = nc.dram_tensor("y_part", [ROWS_PER_CORE, D], F32, kind="ExternalOutput")
    k_d = nc.dram_tensor("k_out", [INST, N, DK], F32, kind="ExternalOutput")
    v_d = nc.dram_tensor("v_out", [INST, N, DK], F32R, kind="ExternalOutput")

    # A2A bounce buffers: shard s of a2a_in goes to core s.
    a2a_in = nc.dram_tensor("a2a_in", [NC, HPC * DK, ROWS_PER_CORE], F32, kind="Internal")
    a2a_out = nc.dram_tensor("a2a_out", [NC, HPC * DK, ROWS_PER_CORE], F32, kind="Internal")

    xT_t = xT_d.rearrange("(t p) n -> p t n", p=128)       # [128, 16, 4096]
    wqT_t = wqT_d.rearrange("(t p) f -> p t f", p=128)     # [128, 16, 256]
    wkT_t = wkT_d.rearrange("(t p) f -> p t f", p=128)
    wvT_t = wvT_d.rearrange("(t p) f -> p t f", p=128)

    with tile.TileContext(nc) as tc:
        with tc.tile_pool(name="const", bufs=1) as const_p, \
             tc.tile_pool(name="pers1", bufs=1) as pers1, \
             tc.tile_pool(name="vnp", bufs=1) as vnp:
            # constants
            ident = const_p.tile([128, 128], F32, tag="ident")
            make_identity(nc, ident[:])
            cmask = const_p.tile([128, 128], F32, tag="cmask")
            make_causal_mask(nc, cmask[:], mask_val=MASK_VAL)
            ident_bf = const_p.tile([128, 128], BF16, tag="ident_bf")
            make_identity(nc, ident_bf[:])

            QT = [pers1.tile([128, N], F32R, tag=f"qt{i}", name=f"qt{i}") for i in range(INST)]
            KT = [pers1.tile([128, N], F32R, tag=f"kt{i}", name=f"kt{i}") for i in range(INST)]
            VN = [vnp.tile([128, ITILES, DK], F32R, tag=f"vn{i}", name=f"vn{i}") for i in range(INST)]

            # ---------------- phase 1: QKV projections (+V transpose inline) --------
            with tc.tile_pool(name="p1sb", bufs=2) as p1sb, \
                 tc.tile_pool(name="p1w", bufs=1) as p1w, \
                 tc.tile_pool(name="p1ps", bufs=3, space="PSUM") as p1ps, \
                 tc.tile_pool(name="tps", bufs=2, space="PSUM") as tps, \
                 tc.tile_pool(name="vtp", bufs=2) as vtp:
                wq_sb = p1w.tile([128, NT, HPC * DK], F32R, tag="wq")
                wk_sb = p1w.tile([128, NT, HPC * DK], F32R, tag="wk")
                wv_sb = p1w.tile([128, NT, HPC * DK], F32R, tag="wv")
                nc.sync.dma_start(wq_sb[:], wqT_t)
                nc.sync.dma_start(wk_sb[:], wkT_t)
                nc.sync.dma_start(wv_sb[:], wvT_t)

                for nt in range(BN // XC):
                    b = (nt * XC) // N
                    col = (nt * XC) % N
                    x_sb = p1sb.tile([128, NT, XC], F32R, tag="x")
                    nc.sync.dma_start(x_sb[:], xT_t[:, :, nt * XC:(nt + 1) * XC])
                    for w_sb, kind in ((wq_sb, "q"), (wk_sb, "k"), (wv_sb, "v")):
                        for hh in range(HPC):
                            inst = b * HPC + hh
                            ps = p1ps.tile([128, XC], F32, tag="p1")
                            for kt in range(NT):
                                nc.tensor.matmul(
                                    ps[:],
                                    w_sb[:, kt, hh * DK:(hh + 1) * DK],
                                    x_sb[:, kt, :],
                                    start=(kt == 0), stop=(kt == NT - 1),
                                )
                            if kind == "q":
                                nc.any.tensor_copy(QT[inst][:, col:col + XC], ps[:])
                            elif kind == "k":
                                nc.any.tensor_copy(KT[inst][:, col:col + XC], ps[:])
                            else:
                                # V: copy to sbuf, transpose to natural layout now
                                vtmp = vtp.tile([128, XC], F32, tag="vtmp")
                                nc.any.tensor_copy(vtmp[:], ps[:])
                                tp = tps.tile([128, XC], F32, tag="tp")
                                for q in range(XC // 128):
                                    nc.tensor.transpose(
                                        tp[:, q * 128:(q + 1) * 128],
                                        vtmp[:, q * 128:(q + 1) * 128],
                                        ident[:],
                                    )
                                jt0 = col // 128
                                nc.any.tensor_copy(
                                    VN[inst][:, jt0:jt0 + XC // 128, :],
                                    tp[:].rearrange("p (q f) -> p q f", f=128))

                for i in range(INST):
                    nc.sync.dma_start(
                        v_d[i].rearrange("(t p) f -> p t f", p=128), VN[i][:])

            # ---------------- phase 1.5: RoPE on QT, KT ----------------
            # roped = cos_full * t + sin_signed * partner_swap(t)
            with tc.tile_pool(name="ropec", bufs=1) as ropec, \
                 tc.tile_pool(name="tcp", bufs=2) as tcp:
                cos_sb = ropec.tile([128, N], F32, tag="cos")
                sin_sb = ropec.tile([128, N], F32, tag="sin")
                nc.sync.dma_start(cos_sb[:], cosT_d[:])
                nc.sync.dma_start(sin_sb[:], sinT_d[:])
                for i in range(INST):
                    for t in (QT[i], KT[i]):
                        tp_sw = tcp.tile([128, N], F32R, tag="ropeP")
                        nc.sync.dma_start(tp_sw[0:64, :], t[64:128, :])
                        nc.sync.dma_start(tp_sw[64:128, :], t[0:64, :])
                        ta = tcp.tile([128, N], F32, tag="ropeA")
                        tb = tcp.tile([128, N], F32, tag="ropeB")
                        nc.vector.tensor_mul(ta[:], t[:].bitcast(F32), cos_sb[:])
                        nc.vector.tensor_mul(tb[:], tp_sw[:].bitcast(F32), sin_sb[:])
                        nc.vector.tensor_add(t[:], ta[:], tb[:])

            # ---------------- phase 2: attention per instance ----------------
            with tc.tile_pool(name="attp", bufs=1) as attp, \
                 tc.tile_pool(name="sps", bufs=1, space="PSUM") as sps, \
                 tc.tile_pool(name="ptps", bufs=2, space="PSUM") as ptps, \
                 tc.tile_pool(name="pvps", bufs=2, space="PSUM") as pvps, \
                 tc.tile_pool(name="p2sb", bufs=2) as p2sb, \
                 tc.tile_pool(name="ptsb", bufs=1) as ptsb:
                ATT = [attp.tile([128, N], F32, tag=f"att{i}", name=f"att{i}") for i in range(INST)]
                for i in range(INST):
                    for ir in range(4):
                        pt_sb = ptsb.tile([128, ITILES, 512], F32R, tag="pt")
                        for itl in range(4):
                            it = ir * 4 + itl
                            jmax = (it + 1) * 128
                            s_ps = sps.tile([128, N], F32, tag="s")
                            for jc in range((jmax + 511) // 512):
                                w = min(512, jmax - jc * 512)
                                nc.tensor.matmul(
                                    s_ps[:, jc * 512:jc * 512 + w],
                                    QT[i][:, it * 128:(it + 1) * 128],
                                    KT[i][:, jc * 512:jc * 512 + w],
                                    start=True, stop=True,
                                )
                            nc.vector.tensor_add(
                                s_ps[:, jmax - 128:jmax],
                                s_ps[:, jmax - 128:jmax],
                                cmask[:])
                            p_sb = p2sb.tile([128, N], BF16, tag="p")
                            sums = p2sb.tile([128, 1], F32, tag="sums")
                            nc.scalar.activation(
                                p_sb[:, :jmax], s_ps[:, :jmax],
                                mybir.ActivationFunctionType.Exp,
                                accum_out=sums[:])
                            rec = p2sb.tile([128, 1], F32, tag="rec")
                            nc.vector.reciprocal(rec[:], sums[:])
                            diag = p2sb.tile([128, 128], BF16, tag="diag")
                            nc.vector.tensor_scalar_mul(diag[:], ident_bf[:], rec[:])
                            # zero the causally-masked (never-written) segments
                            njt = 4 * (ir + 1)
                            for jt_z in range(it + 1, njt):
                                nc.vector.memset(
                                    pt_sb[:, jt_z, itl * 128:(itl + 1) * 128].bitcast(F32),
                                    0.0)
                            # transpose P tiles (scaled by diag) into pt_sb
                            for jg in range((it + 1 + 3) // 4):
                                q_n = min(4, it + 1 - jg * 4)
                                ps = ptps.tile([128, 512], F32, tag="ptp")
                                for q in range(q_n):
                                    jt = jg * 4 + q
                                    nc.tensor.matmul(
                                        ps[:, q * 128:(q + 1) * 128],
                                        p_sb[:, jt * 128:(jt + 1) * 128],
                                        diag[:],
                                        start=True, stop=True,
                                    )
                                nc.any.tensor_copy(
                                    pt_sb[:, jg * 4:jg * 4 + q_n, itl * 128:(itl + 1) * 128],
                                    ps[:, :q_n * 128].rearrange("p (q f) -> p q f", f=128))
                        # PV for this i-range
                        pv = pvps.tile([128, 512], F32, tag="pv")
                        njt = 4 * (ir + 1)
                        for jt in range(njt):
                            nc.tensor.matmul(
                                pv[:],
                                VN[i][:, jt, :],
                                pt_sb[:, jt, :],
                                start=(jt == 0), stop=(jt == njt - 1),
                            )
                        nc.any.tensor_copy(ATT[i][:, ir * 512:(ir + 1) * 512], pv[:])

                # K natural-layout output (post-rope), overlaps with attention/A2A
                for i in range(INST):
                    for jg in range(ITILES // 4):
                        ps = ptps.tile([128, 512], F32, tag="ptp")
                        for q in range(4):
                            jt = jg * 4 + q
                            nc.tensor.transpose(
                                ps[:, q * 128:(q + 1) * 128],
                                KT[i][:, jt * 128:(jt + 1) * 128].bitcast(F32),
                                ident[:],
                            )
                        kb = p2sb.tile([128, 4, DK], F32, tag="kn")
                        # src col = blk*64 + t  ->  dst col = 2*t + blk
                        src = ps[:].rearrange("p (q blk t) -> p q blk t", q=4, blk=2)
                        dst = kb[:].rearrange("p q (h two) -> p q two h", two=2)
                        nc.any.tensor_copy(dst, src)
                        nc.sync.dma_start(
                            k_d[i].rearrange("(t p) f -> p t f", p=128)[:, jg * 4:jg * 4 + 4, :],
                            kb[:])

                # ---------------- A2A: reshard heads -> sequence ----------------
                for s in range(NC):
                    b = s // 4
                    col = (s % 4) * 512
                    for hh in range(HPC):
                        nc.sync.dma_start(
                            a2a_in[s, hh * DK:(hh + 1) * DK, :],
                            ATT[b * HPC + hh][:, col:col + 512])
                nc.gpsimd.collective_compute(
                    "AllToAll", mybir.AluOpType.bypass,
                    ins=[a2a_in[:]], outs=[a2a_out[:]],
                    replica_groups=[list(range(NC))],
                )

        # ---------------- phase 3: out_proj on my 512 rows ----------------
        with tc.tile_pool(name="p3sb", bufs=1) as p3sb, \
             tc.tile_pool(name="p3w", bufs=2) as p3w, \
             tc.tile_pool(name="p3y", bufs=2) as p3y, \
             tc.tile_pool(name="yps", bufs=2, space="PSUM") as yps:
            att_f = p3sb.tile([128, NT, 512], F32R, tag="attf")
            nc.sync.dma_start(
                att_f[:],
                a2a_out.rearrange("g (t p) n -> p (g t) n", p=128).bitcast(F32R))
            for oh in range(2):
                wo_sb = p3w.tile([128, NT, 1024], F32R, tag="wo")
                nc.sync.dma_start(
                    wo_sb[:],
                    woT_d.rearrange("(t p) o -> p t o", p=128)[:, :, oh * 1024:(oh + 1) * 1024])
                for ntl in range(4):
                    y_ps = yps.tile([128, 1024], F32, tag="y")
                    for kt in range(NT):
                        for half in range(2):
                            nc.tensor.matmul(
                                y_ps[:, half * 512:(half + 1) * 512],
                                att_f[:, kt, ntl * 128:(ntl + 1) * 128],
                                wo_sb[:, kt, half * 512:(half + 1) * 512],
                                start=(kt == 0), stop=(kt == NT - 1),
                            )
                    yh = p3y.tile([128, 1024], F32, tag="yh")
                    nc.any.tensor_copy(yh[:], y_ps[:])
                    nc.sync.dma_start(
                        y_d[ntl * 128:(ntl + 1) * 128, oh * 1024:(oh + 1) * 1024],
                        yh[:])
    nc.finalize()
    return nc


_CACHE = {}


def _get_nc():
    if "nc" not in _CACHE:
        _CACHE["nc"] = build_kernel()
    return _CACHE["nc"]


def _rope_cache_np():
    half = DK // 2
    inv_freq = 1.0 / (10000.0 ** (np.arange(half, dtype=np.float32) / half))
    ang = np.arange(N, dtype=np.float32)[:, None] * inv_freq[None, :]
    return np.cos(ang).astype(np.float32), np.sin(ang).astype(np.float32)


def _make_in_maps(x, wq, wk, wv, wo):
    xT = np.ascontiguousarray(x.reshape(BN, D).T)           # [D, BN]
    woT = np.ascontiguousarray(wo.T)                        # [D, D]
    cos, sin = _rope_cache_np()
    # full-partition rope tables for the evens/odds-block permuted layout:
    # rows 0-63 (x1 block): cos, -sin ; rows 64-127 (x2 block): cos, +sin
    cosT = np.ascontiguousarray(np.concatenate([cos.T, cos.T], axis=0))   # [128, N]
    sinT = np.ascontiguousarray(np.concatenate([-sin.T, sin.T], axis=0))  # [128, N]

    # rope permutation within each head: evens then odds
    perm = np.concatenate([np.arange(0, DK, 2), np.arange(1, DK, 2)])
    invs = np.float32(1.0 / np.sqrt(np.float32(DK)))

    in_maps = []
    for c in range(NC):
        rows = slice(c * HPC * DK, (c + 1) * HPC * DK)
        wq_c = wq[rows].reshape(HPC, DK, D)[:, perm, :].reshape(HPC * DK, D) * invs
        wk_c = wk[rows].reshape(HPC, DK, D)[:, perm, :].reshape(HPC * DK, D)
        wv_c = wv[rows]
        in_maps.append({
            "xT": xT,
            "wqT": np.ascontiguousarray(wq_c.T),
            "wkT": np.ascontiguousarray(wk_c.T),
            "wvT": np.ascontiguousarray(wv_c.T),
            "woT": woT,
            "cosT": cosT,
            "sinT": sinT,
        })
    return in_maps


def _assemble(results):
    out = np.empty((BN, D), dtype=np.float32)
    K = np.empty((B, H, N, DK), dtype=np.float32)
    V = np.empty((B, H, N, DK), dtype=np.float32)
    for c in range(NC):
        out[c * ROWS_PER_CORE:(c + 1) * ROWS_PER_CORE] = results[c]["y_part"]
        ko = results[c]["k_out"]
        vo = results[c]["v_out"]
        for b in range(B):
            for hh in range(HPC):
                K[b, HPC * c + hh] = ko[b * HPC + hh]
                V[b, HPC * c + hh] = vo[b * HPC + hh]
    return out.reshape(B, N, D), K, V


def kernel(x, wq, wk, wv, wo):
    nc = _get_nc()
    x = np.asarray(x, dtype=np.float32)
    wq = np.asarray(wq, dtype=np.float32)
    wk = np.asarray(wk, dtype=np.float32)
    wv = np.asarray(wv, dtype=np.float32)
    wo = np.asarray(wo, dtype=np.float32)
    in_maps = _make_in_maps(x, wq, wk, wv, wo)
    res = run_bass_kernel_spmd(nc, in_maps, core_ids=list(range(NC)))
    return _assemble(res.results)


# revision 15
# speedup vs baseline: 1.0400x; 1.0400x over previous
"""Distributed MultiHeadAttention (B=2, N=2048, D=2048, H=16, dk=128) on 8 TRN2 cores.

Sharding: head-parallel (2 heads/core, both batches => 4 (b,h) instances/core).
All projections/attention computed in transposed layout ([dim, seq]).
AllToAll (bf16) reshards attention outputs so out_proj is sequence-parallel
(each core produces 512 rows of the final output).
"""
import numpy as np
import ml_dtypes

import concourse.bacc as bacc
import concourse.mybir as mybir
import concourse.tile as tile
from concourse.bass_utils import run_bass_kernel_spmd
from concourse.masks import make_identity, make_causal_mask

F32 = mybir.dt.float32
F32R = mybir.dt.float32r
BF16 = mybir.dt.bfloat16

NC = 8          # cores
B = 2
N = 2048
D = 2048
H = 16
DK = 128
HPC = H // NC   # heads per core = 2
INST = B * HPC  # (b, h) instances per core = 4
BN = B * N      # 4096 flattened rows
ROWS_PER_CORE = BN // NC  # 512
NT = 16         # 128-row tiles in D (k-tiles)
ITILES = N // 128  # 16 i-tiles per instance
XC = 512        # x chunk width in phase 1
MASK_VAL = -1e30


def build_kernel():
    nc = bacc.Bacc("TRN2", num_devices=NC)

    # ---------------- DRAM I/O ----------------
    xT_d = nc.dram_tensor("xT", [D, BN], F32R, kind="ExternalInput")
    wqT_d = nc.dram_tensor("wqT", [D, HPC * DK], F32R, kind="ExternalInput")
    wkT_d = nc.dram_tensor("wkT", [D, HPC * DK], F32R, kind="ExternalInput")
    wvT_d = nc.dram_tensor("wvT", [D, HPC * DK], F32R, kind="ExternalInput")
    woT_d = nc.dram_tensor("woT", [D, D], BF16, kind="ExternalInput")
    cosT_d = nc.dram_tensor("cosT", [128, N], F32, kind="ExternalInput")
    sinT_d = nc.dram_tensor("sinT", [128, N], F32, kind="ExternalInput")

    y_d = nc.dram_tensor("y_part", [ROWS_PER_CORE, D], F32, kind="ExternalOutput")
    k_d = nc.dram_tensor("k_out", [INST, N, DK], F32, kind="ExternalOutput")
    v_d = nc.dram_tensor("v_out", [INST, N, DK], F32, kind="ExternalOutput")

    # A2A bounce buffers (bf16): shard s of a2a_in goes to core s.
    a2a_in = nc.dram_tensor("a2a_in", [NC, HPC * DK, ROWS_PER_CORE], BF16, kind="Internal")
    a2a_out = nc.dram_tensor("a2a_out", [NC, HPC * DK, ROWS_PER_CORE], BF16, kind="Internal")

    xT_t = xT_d.rearrange("(t p) n -> p t n", p=128)       # [128, 16, 4096]
    wqT_t = wqT_d.rearrange("(t p) f -> p t f", p=128)     # [128, 16, 256]
    wkT_t = wkT_d.rearrange("(t p) f -> p t f", p=128)
    wvT_t = wvT_d.rearrange("(t p) f -> p t f", p=128)

    with tile.TileContext(nc) as tc:
        with tc.tile_pool(name="const", bufs=1) as const_p, \
             tc.tile_pool(name="pers1", bufs=1) as pers1:
            # constants
            ident = const_p.tile([128, 128], F32, tag="ident")
            make_identity(nc, ident[:])
            cmask = const_p.tile([128, 128], F32, tag="cmask")
            make_causal_mask(nc, cmask[:], mask_val=MASK_VAL)
            ident_bf = const_p.tile([128, 128], BF16, tag="ident_bf")
            make_identity(nc, ident_bf[:])

            QT = [pers1.tile([128, N], F32R, tag=f"qt{i}", name=f"qt{i}") for i in range(INST)]
            KT = [pers1.tile([128, N], F32R, tag=f"kt{i}", name=f"kt{i}") for i in range(INST)]

            # ---------------- phase 1: QKV projections (+V out inline) --------
            with tc.tile_pool(name="p1sb", bufs=2) as p1sb, \
                 tc.tile_pool(name="p1w", bufs=1) as p1w, \
                 tc.tile_pool(name="p1ps", bufs=3, space="PSUM") as p1ps, \
                 tc.tile_pool(name="tps", bufs=2, space="PSUM") as tps, \
                 tc.tile_pool(name="vtp", bufs=2) as vtp:
                wq_sb = p1w.tile([128, NT, HPC * DK], F32R, tag="wq")
                wk_sb = p1w.tile([128, NT, HPC * DK], F32R, tag="wk")
                wv_sb = p1w.tile([128, NT, HPC * DK], F32R, tag="wv")
                nc.sync.dma_start(wq_sb[:], wqT_t)
                nc.sync.dma_start(wk_sb[:], wkT_t)
                nc.sync.dma_start(wv_sb[:], wvT_t)

                for nt in range(BN // XC):
                    b = (nt * XC) // N
                    col = (nt * XC) % N
                    x_sb = p1sb.tile([128, NT, XC], F32R, tag="x")
                    nc.sync.dma_start(x_sb[:], xT_t[:, :, nt * XC:(nt + 1) * XC])
                    for w_sb, kind in ((wq_sb, "q"), (wk_sb, "k"), (wv_sb, "v")):
                        for hh in range(HPC):
                            inst = b * HPC + hh
                            ps = p1ps.tile([128, XC], F32, tag="p1")
                            for kt in range(NT):
                                nc.tensor.matmul(
                                    ps[:],
                                    w_sb[:, kt, hh * DK:(hh + 1) * DK],
                                    x_sb[:, kt, :],
                                    start=(kt == 0), stop=(kt == NT - 1),
                                )
                            if kind == "q":
                                nc.any.tensor_copy(QT[inst][:, col:col + XC], ps[:])
                            elif kind == "k":
                                nc.any.tensor_copy(KT[inst][:, col:col + XC], ps[:])
                            else:
                                # V: copy to sbuf, transpose to natural layout,
                                # write straight to v_out DRAM
                                vtmp = vtp.tile([128, XC], F32, tag="vtmp")
                                nc.any.tensor_copy(vtmp[:], ps[:])
                                tp = tps.tile([128, XC], F32, tag="tp")
                                for q in range(XC // 128):
                                    nc.tensor.transpose(
                                        tp[:, q * 128:(q + 1) * 128],
                                        vtmp[:, q * 128:(q + 1) * 128],
                                        ident[:],
                                    )
                                vns = vtp.tile([128, XC // 128, DK], F32, tag="vns")
                                nc.any.tensor_copy(
                                    vns[:],
                                    tp[:].rearrange("p (q f) -> p q f", f=128))
                                jt0 = col // 128
                                nc.sync.dma_start(
                                    v_d[inst].rearrange("(t p) f -> p t f", p=128)[:, jt0:jt0 + XC // 128, :],
                                    vns[:])

            # ---------------- phase 1.5: RoPE on QT, KT ----------------
            # roped = cos_full * t + sin_signed * partner_swap(t)
            with tc.tile_pool(name="ropec", bufs=1) as ropec, \
                 tc.tile_pool(name="tcp", bufs=2) as tcp:
                cos_sb = ropec.tile([128, N], F32, tag="cos")
                sin_sb = ropec.tile([128, N], F32, tag="sin")
                nc.sync.dma_start(cos_sb[:], cosT_d[:])
                nc.sync.dma_start(sin_sb[:], sinT_d[:])
                for i in range(INST):
                    for t in (QT[i], KT[i]):
                        tp_sw = tcp.tile([128, N], F32R, tag="ropeP")
                        nc.sync.dma_start(tp_sw[0:64, :], t[64:128, :])
                        nc.sync.dma_start(tp_sw[64:128, :], t[0:64, :])
                        ta = tcp.tile([128, N], F32, tag="ropeA")
                        tb = tcp.tile([128, N], F32, tag="ropeB")
                        nc.vector.tensor_mul(ta[:], t[:].bitcast(F32), cos_sb[:])
                        nc.vector.tensor_mul(tb[:], tp_sw[:].bitcast(F32), sin_sb[:])
                        nc.vector.tensor_add(t[:], ta[:], tb[:])

            # ---------------- phase 2: attention per instance ----------------
            with tc.tile_pool(name="attp", bufs=1) as attp, \
                 tc.tile_pool(name="vnbp", bufs=1) as vnbp, \
                 tc.tile_pool(name="sps", bufs=1, space="PSUM") as sps, \
                 tc.tile_pool(name="ptps", bufs=2, space="PSUM") as ptps, \
                 tc.tile_pool(name="pvps", bufs=2, space="PSUM") as pvps, \
                 tc.tile_pool(name="p2sb", bufs=2) as p2sb, \
                 tc.tile_pool(name="ptsb", bufs=2) as ptsb:
                ATT = [attp.tile([128, N], BF16, tag=f"att{i}", name=f"att{i}") for i in range(INST)]
                # reload V in bf16 for the PV matmul (gpsimd DMA casts f32->bf16)
                VNB = [vnbp.tile([128, ITILES, DK], BF16, tag=f"vnb{i}", name=f"vnb{i}")
                       for i in range(INST)]
                for i in range(INST):
                    nc.gpsimd.dma_start(
                        VNB[i][:], v_d[i].rearrange("(t p) f -> p t f", p=128))

                for i in range(INST):
                    for ir in range(4):
                        pt_sb = ptsb.tile([128, ITILES, 512], BF16, tag="pt")
                        for itl in range(4):
                            it = ir * 4 + itl
                            jmax = (it + 1) * 128
                            s_ps = sps.tile([128, N], F32, tag="s")
                            for jc in range((jmax + 511) // 512):
                                w = min(512, jmax - jc * 512)
                                nc.tensor.matmul(
                                    s_ps[:, jc * 512:jc * 512 + w],
                                    QT[i][:, it * 128:(it + 1) * 128],
                                    KT[i][:, jc * 512:jc * 512 + w],
                                    start=True, stop=True,
                                )
                            nc.vector.tensor_add(
                                s_ps[:, jmax - 128:jmax],
                                s_ps[:, jmax - 128:jmax],
                                cmask[:])
                            p_sb = p2sb.tile([128, N], BF16, tag="p")
                            sums = p2sb.tile([128, 1], F32, tag="sums")
                            nc.scalar.activation(
                                p_sb[:, :jmax], s_ps[:, :jmax],
                                mybir.ActivationFunctionType.Exp,
                                accum_out=sums[:])
                            rec = p2sb.tile([128, 1], F32, tag="rec")
                            nc.vector.reciprocal(rec[:], sums[:])
                            diag = p2sb.tile([128, 128], BF16, tag="diag")
                            nc.vector.tensor_scalar_mul(diag[:], ident_bf[:], rec[:])
                            # zero the causally-masked (never-written) segments
                            njt = 4 * (ir + 1)
                            for jt_z in range(it + 1, njt):
                                nc.vector.memset(
                                    pt_sb[:, jt_z, itl * 128:(itl + 1) * 128], 0.0)
                            # scaled transpose: P_tile.T @ diag(rec), bf16
                            for jg in range((it + 1 + 3) // 4):
                                q_n = min(4, it + 1 - jg * 4)
                                ps = ptps.tile([128, 512], F32, tag="ptp")
                                for q in range(q_n):
                                    jt = jg * 4 + q
                                    nc.tensor.matmul(
                                        ps[:, q * 128:(q + 1) * 128],
                                        p_sb[:, jt * 128:(jt + 1) * 128],
                                        diag[:],
                                        start=True, stop=True,
                                    )
                                nc.any.tensor_copy(
                                    pt_sb[:, jg * 4:jg * 4 + q_n, itl * 128:(itl + 1) * 128],
                                    ps[:, :q_n * 128].rearrange("p (q f) -> p q f", f=128))
                        # PV for this i-range (bf16)
                        pv = pvps.tile([128, 512], F32, tag="pv")
                        njt = 4 * (ir + 1)
                        for jt in range(njt):
                            nc.tensor.matmul(
                                pv[:],
                                VNB[i][:, jt, :],
                                pt_sb[:, jt, :],
                                start=(jt == 0), stop=(jt == njt - 1),
                            )
                        nc.any.tensor_copy(ATT[i][:, ir * 512:(ir + 1) * 512], pv[:])

                # K natural-layout output (post-rope), overlaps with attention/A2A
                for i in range(INST):
                    for jg in range(ITILES // 4):
                        ps = ptps.tile([128, 512], F32, tag="ptp")
                        for q in range(4):
                            jt = jg * 4 + q
                            nc.tensor.transpose(
                                ps[:, q * 128:(q + 1) * 128],
                                KT[i][:, jt * 128:(jt + 1) * 128].bitcast(F32),
                                ident[:],
                            )
                        kb = p2sb.tile([128, 4, DK], F32, tag="kn")
                        # src col = blk*64 + t  ->  dst col = 2*t + blk
                        src = ps[:].rearrange("p (q blk t) -> p q blk t", q=4, blk=2)
                        dst = kb[:].rearrange("p q (h two) -> p q two h", two=2)
                        nc.any.tensor_copy(dst, src)
                        nc.sync.dma_start(
                            k_d[i].rearrange("(t p) f -> p t f", p=128)[:, jg * 4:jg * 4 + 4, :],
                            kb[:])

                # ---------------- A2A: reshard heads -> sequence ----------------
                for s in range(NC):
                    b = s // 4
                    col = (s % 4) * 512
                    for hh in range(HPC):
                        nc.sync.dma_start(
                            a2a_in[s, hh * DK:(hh + 1) * DK, :],
                            ATT[b * HPC + hh][:, col:col + 512])
                nc.gpsimd.collective_compute(
                    "AllToAll", mybir.AluOpType.bypass,
                    ins=[a2a_in[:]], outs=[a2a_out[:]],
                    replica_groups=[list(range(NC))],
                )

        # ---------------- phase 3: out_proj on my 512 rows ----------------
        with tc.tile_pool(name="p3sb", bufs=1) as p3sb, \
             tc.tile_pool(name="p3w", bufs=2) as p3w, \
             tc.tile_pool(name="p3y", bufs=2) as p3y, \
             tc.tile_pool(name="yps", bufs=2, space="PSUM") as yps:
            att_f = p3sb.tile([128, NT, 512], BF16, tag="attf")
            nc.sync.dma_start(
                att_f[:],
                a2a_out.rearrange("g (t p) n -> p (g t) n", p=128))
            for oh in range(2):
                wo_sb = p3w.tile([128, NT, 1024], BF16, tag="wo")
                nc.sync.dma_start(
                    wo_sb[:],
                    woT_d.rearrange("(t p) o -> p t o", p=128)[:, :, oh * 1024:(oh + 1) * 1024])
                for ntl in range(4):
                    y_ps = yps.tile([128, 1024], F32, tag="y")
                    for kt in range(NT):
                        for half in range(2):
                            nc.tensor.matmul(
                                y_ps[:, half * 512:(half + 1) * 512],
                                att_f[:, kt, ntl * 128:(ntl + 1) * 128],
                                wo_sb[:, kt, half * 512:(half + 1) * 512],
                                start=(kt == 0), stop=(kt == NT - 1),
                            )
                    yh = p3y.tile([128, 1024], F32, tag="yh")
                    nc.any.tensor_copy(yh[:], y_ps[:])
                    nc.sync.dma_start(
                        y_d[ntl * 128:(ntl + 1) * 128, oh * 1024:(oh + 1) * 1024],
                        yh[:])
    nc.finalize()
    return nc


_CACHE = {}


def _get_nc():
    if "nc" not in _CACHE:
        _CACHE["nc"] = build_kernel()
    return _CACHE["nc"]


def _rope_cache_np():
    half = DK // 2
    inv_freq = 1.0 / (10000.0 ** (np.arange(half, dtype=np.float32) / half))
    ang = np.arange(N, dtype=np.float32)[:, None] * inv_freq[None, :]
    return np.cos(ang).astype(np.float32), np.sin(ang).astype(np.float32)


def _make_in_maps(x, wq, wk, wv, wo):
    xT = np.ascontiguousarray(x.reshape(BN, D).T)           # [D, BN]
    woT = np.ascontiguousarray(wo.T.astype(ml_dtypes.bfloat16))  # [D, D] bf16
    cos, sin = _rope_cache_np()
    # full-partition rope tables for the evens/odds-block permuted layout:
    # rows 0-63 (x1 block): cos, -sin ; rows 64-127 (x2 block): cos, +sin
    cosT = np.ascontiguousarray(np.concatenate([cos.T, cos.T], axis=0))   # [128, N]
    sinT = np.ascontiguousarray(np.concatenate([-sin.T, sin.T], axis=0))  # [128, N]

    # rope permutation within each head: evens then odds
    perm = np.concatenate([np.arange(0, DK, 2), np.arange(1, DK, 2)])
    invs = np.float32(1.0 / np.sqrt(np.float32(DK)))

    in_maps = []
    for c in range(NC):
        rows = slice(c * HPC * DK, (c + 1) * HPC * DK)
        wq_c = wq[rows].reshape(HPC, DK, D)[:, perm, :].reshape(HPC * DK, D) * invs
        wk_c = wk[rows].reshape(HPC, DK, D)[:, perm, :].reshape(HPC * DK, D)
        wv_c = wv[rows]
        in_maps.append({
            "xT": xT,
            "wqT": np.ascontiguousarray(wq_c.T),
            "wkT": np.ascontiguousarray(wk_c.T),
            "wvT": np.ascontiguousarray(wv_c.T),
            "woT": woT,
            "cosT": cosT,
            "sinT": sinT,
        })
    return in_maps


def _assemble(results):
    out = np.empty((BN, D), dtype=np.float32)
    K = np.empty((B, H, N, DK), dtype=np.float32)
    V = np.empty((B, H, N, DK), dtype=np.float32)
    for c in range(NC):
        out[c * ROWS_PER_CORE:(c + 1) * ROWS_PER_CORE] = results[c]["y_part"]
        ko = results[c]["k_out"]
        vo = results[c]["v_out"]
        for b in range(B):
            for hh in range(HPC):
                K[b, HPC * c + hh] = ko[b * HPC + hh]
                V[b, HPC * c + hh] = vo[b * HPC + hh]
    return out.reshape(B, N, D), K, V


def kernel(x, wq, wk, wv, wo):
    nc = _get_nc()
    x = np.asarray(x, dtype=np.float32)
    wq = np.asarray(wq, dtype=np.float32)
    wk = np.asarray(wk, dtype=np.float32)
    wv = np.asarray(wv, dtype=np.float32)
    wo = np.asarray(wo, dtype=np.float32)
    in_maps = _make_in_maps(x, wq, wk, wv, wo)
    res = run_bass_kernel_spmd(nc, in_maps, core_ids=list(range(NC)))
    return _assemble(res.results)


# revision 17
# speedup vs baseline: 1.3750x; 1.3221x over previous
"""Distributed MultiHeadAttention (B=2, N=2048, D=2048, H=16, dk=128) on 8 TRN2 cores.

Sharding: head-parallel (2 heads/core, both batches => 4 (b,h) instances/core).
All projections/attention computed in transposed layout ([dim, seq]), bf16
compute with fp32 PSUM accumulation. Two per-head-slot AllToAlls (bf16)
reshard attention outputs so out_proj is sequence-parallel (each core
produces 512 rows of the final output); the first A2A overlaps with the
second half of attention.
"""
import numpy as np
import ml_dtypes

import concourse.bacc as bacc
import concourse.mybir as mybir
import concourse.tile as tile
from concourse.bass_utils import run_bass_kernel_spmd
from concourse.masks import make_identity, make_causal_mask

F32 = mybir.dt.float32
F32R = mybir.dt.float32r
BF16 = mybir.dt.bfloat16

NC = 8          # cores
B = 2
N = 2048
D = 2048
H = 16
DK = 128
HPC = H // NC   # heads per core = 2
INST = B * HPC  # (b, h) instances per core = 4
BN = B * N      # 4096 flattened rows
ROWS_PER_CORE = BN // NC  # 512
NT = 16         # 128-row tiles in D (k-tiles)
ITILES = N // 128  # 16 i-tiles per instance
XC = 512        # x chunk width in phase 1
MASK_VAL = -1e30


def build_kernel():
    nc = bacc.Bacc("TRN2", num_devices=NC)

    # ---------------- DRAM I/O ----------------
    xT_d = nc.dram_tensor("xT", [D, BN], BF16, kind="ExternalInput")
    wqT_d = nc.dram_tensor("wqT", [D, HPC * DK], BF16, kind="ExternalInput")
    wkT_d = nc.dram_tensor("wkT", [D, HPC * DK], BF16, kind="ExternalInput")
    wvT_d = nc.dram_tensor("wvT", [D, HPC * DK], BF16, kind="ExternalInput")
    woT_d = nc.dram_tensor("woT", [D, D], BF16, kind="ExternalInput")
    cosT_d = nc.dram_tensor("cosT", [128, N], BF16, kind="ExternalInput")
    sinT_d = nc.dram_tensor("sinT", [128, N], BF16, kind="ExternalInput")

    y_d = nc.dram_tensor("y_part", [ROWS_PER_CORE, D], F32, kind="ExternalOutput")
    k_d = nc.dram_tensor("k_out", [INST, N, DK], F32, kind="ExternalOutput")
    v_d = nc.dram_tensor("v_out", [INST, N, DK], F32, kind="ExternalOutput")

    # Per-head-slot A2A bounce buffers (bf16): shard s goes to core s.
    a2a_in = [nc.dram_tensor(f"a2a_in{j}", [NC, DK, ROWS_PER_CORE], BF16, kind="Internal")
              for j in range(HPC)]
    a2a_out = [nc.dram_tensor(f"a2a_out{j}", [NC, DK, ROWS_PER_CORE], BF16, kind="Internal")
               for j in range(HPC)]

    xT_t = xT_d.rearrange("(t p) n -> p t n", p=128)       # [128, 16, 4096]
    wqT_t = wqT_d.rearrange("(t p) f -> p t f", p=128)     # [128, 16, 256]
    wkT_t = wkT_d.rearrange("(t p) f -> p t f", p=128)
    wvT_t = wvT_d.rearrange("(t p) f -> p t f", p=128)

    with tile.TileContext(nc) as tc:
        with tc.tile_pool(name="const", bufs=1) as const_p, \
             tc.tile_pool(name="pers1", bufs=1) as pers1:
            # constants
            ident = const_p.tile([128, 128], F32, tag="ident")
            make_identity(nc, ident[:])
            cmask = const_p.tile([128, 128], F32, tag="cmask")
            make_causal_mask(nc, cmask[:], mask_val=MASK_VAL)
            ident_bf = const_p.tile([128, 128], BF16, tag="ident_bf")
            make_identity(nc, ident_bf[:])
            cos_sb = const_p.tile([128, N], BF16, tag="cos")
            sin_sb = const_p.tile([128, N], BF16, tag="sin")
            nc.sync.dma_start(cos_sb[:], cosT_d[:])
            nc.sync.dma_start(sin_sb[:], sinT_d[:])

            QT = [pers1.tile([128, N], BF16, tag=f"qt{i}", name=f"qt{i}") for i in range(INST)]
            KT = [pers1.tile([128, N], BF16, tag=f"kt{i}", name=f"kt{i}") for i in range(INST)]

            def rope(i, tcp):
                # roped = cos_full * t + sin_signed * partner_swap(t)
                for t in (QT[i], KT[i]):
                    tp_sw = tcp.tile([128, N], BF16, tag="ropeP")
                    nc.sync.dma_start(tp_sw[0:64, :], t[64:128, :])
                    nc.sync.dma_start(tp_sw[64:128, :], t[0:64, :])
                    ta = tcp.tile([128, N], BF16, tag="ropeA")
                    tb = tcp.tile([128, N], BF16, tag="ropeB")
                    nc.vector.tensor_mul(ta[:], t[:], cos_sb[:])
                    nc.vector.tensor_mul(tb[:], tp_sw[:], sin_sb[:])
                    nc.vector.tensor_add(t[:], ta[:], tb[:])

            # ---------------- phase 1: QKV projections (+V out inline) --------
            with tc.tile_pool(name="p1sb", bufs=2) as p1sb, \
                 tc.tile_pool(name="p1w", bufs=1) as p1w, \
                 tc.tile_pool(name="p1ps", bufs=3, space="PSUM") as p1ps, \
                 tc.tile_pool(name="tps", bufs=2, space="PSUM") as tps, \
                 tc.tile_pool(name="vtp", bufs=2) as vtp, \
                 tc.tile_pool(name="tcp", bufs=2) as tcp:
                wq_sb = p1w.tile([128, NT, HPC * DK], BF16, tag="wq")
                wk_sb = p1w.tile([128, NT, HPC * DK], BF16, tag="wk")
                wv_sb = p1w.tile([128, NT, HPC * DK], BF16, tag="wv")
                nc.sync.dma_start(wq_sb[:], wqT_t)
                nc.sync.dma_start(wk_sb[:], wkT_t)
                nc.sync.dma_start(wv_sb[:], wvT_t)

                for nt in range(BN // XC):
                    b = (nt * XC) // N
                    col = (nt * XC) % N
                    x_sb = p1sb.tile([128, NT, XC], BF16, tag="x")
                    nc.sync.dma_start(x_sb[:], xT_t[:, :, nt * XC:(nt + 1) * XC])
                    for w_sb, kind in ((wq_sb, "q"), (wk_sb, "k"), (wv_sb, "v")):
                        for hh in range(HPC):
                            inst = b * HPC + hh
                            ps = p1ps.tile([128, XC], F32, tag="p1")
                            for kt in range(NT):
                                nc.tensor.matmul(
                                    ps[:],
                                    w_sb[:, kt, hh * DK:(hh + 1) * DK],
                                    x_sb[:, kt, :],
                                    start=(kt == 0), stop=(kt == NT - 1),
                                )
                            if kind == "q":
                                nc.any.tensor_copy(QT[inst][:, col:col + XC], ps[:])
                            elif kind == "k":
                                nc.any.tensor_copy(KT[inst][:, col:col + XC], ps[:])
                            else:
                                # V: copy to sbuf, transpose to natural layout,
                                # write straight to v_out DRAM
                                vtmp = vtp.tile([128, XC], F32, tag="vtmp")
                                nc.any.tensor_copy(vtmp[:], ps[:])
                                tp = tps.tile([128, XC], F32, tag="tp")
                                for q in range(XC // 128):
                                    nc.tensor.transpose(
                                        tp[:, q * 128:(q + 1) * 128],
                                        vtmp[:, q * 128:(q + 1) * 128],
                                        ident[:],
                                    )
                                vns = vtp.tile([128, XC // 128, DK], F32, tag="vns")
                                nc.any.tensor_copy(
                                    vns[:],
                                    tp[:].rearrange("p (q f) -> p q f", f=128))
                                jt0 = col // 128
                                nc.sync.dma_start(
                                    v_d[inst].rearrange("(t p) f -> p t f", p=128)[:, jt0:jt0 + XC // 128, :],
                                    vns[:])
                    if nt == N // XC - 1:       # b=0 instances complete
                        rope(0, tcp)
                        rope(1, tcp)
                rope(2, tcp)
                rope(3, tcp)

            # ---------------- phase 2: attention per instance ----------------
            # order 0,2,1,3 so head-slot 0 finishes first and its A2A overlaps
            with tc.tile_pool(name="attp", bufs=1) as attp, \
                 tc.tile_pool(name="vnbp", bufs=1) as vnbp, \
                 tc.tile_pool(name="sps", bufs=2, space="PSUM") as sps, \
                 tc.tile_pool(name="ptps", bufs=2, space="PSUM") as ptps, \
                 tc.tile_pool(name="pvps", bufs=2, space="PSUM") as pvps, \
                 tc.tile_pool(name="p2sb", bufs=2) as p2sb, \
                 tc.tile_pool(name="ptsb", bufs=2) as ptsb:
                ATT = [attp.tile([128, N], BF16, tag=f"att{i}", name=f"att{i}") for i in range(INST)]
                # reload V in bf16 for the PV matmul (gpsimd DMA casts f32->bf16)
                VNB = [vnbp.tile([128, ITILES, DK], BF16, tag=f"vnb{i}", name=f"vnb{i}")
                       for i in range(INST)]
                for i in range(INST):
                    nc.gpsimd.dma_start(
                        VNB[i][:], v_d[i].rearrange("(t p) f -> p t f", p=128))

                def stage_a2a(j):
                    for s in range(NC):
                        b = s // 4
                        col = (s % 4) * 512
                        nc.sync.dma_start(
                            a2a_in[j][s],
                            ATT[b * HPC + j][:, col:col + 512])
                    nc.gpsimd.collective_compute(
                        "AllToAll", mybir.AluOpType.bypass,
                        ins=[a2a_in[j][:]], outs=[a2a_out[j][:]],
                        replica_groups=[list(range(NC))],
                    )

                for i in (0, 2, 1, 3):
                    for ir in range(4):
                        pt_sb = ptsb.tile([128, ITILES, 512], BF16, tag="pt")
                        for itl in range(4):
                            it = ir * 4 + itl
                            jmax = (it + 1) * 128
                            nhalf = (jmax + 1023) // 1024
                            sums = [None, None]
                            p_sb = p2sb.tile([128, N], BF16, tag="p")
                            for hf in range(nhalf):
                                h0 = hf * 1024
                                hw = min(1024, jmax - h0)
                                s_ps = sps.tile([128, 1024], F32, tag="s")
                                for jc in range((hw + 511) // 512):
                                    w = min(512, hw - jc * 512)
                                    nc.tensor.matmul(
                                        s_ps[:, jc * 512:jc * 512 + w],
                                        QT[i][:, it * 128:(it + 1) * 128],
                                        KT[i][:, h0 + jc * 512:h0 + jc * 512 + w],
                                        start=True, stop=True,
                                    )
                                if h0 + hw == jmax:  # diagonal block lives here
                                    nc.vector.tensor_add(
                                        s_ps[:, hw - 128:hw],
                                        s_ps[:, hw - 128:hw],
                                        cmask[:])
                                sm = p2sb.tile([128, 1], F32, tag=f"sums{hf}")
                                nc.scalar.activation(
                                    p_sb[:, h0:h0 + hw], s_ps[:, :hw],
                                    mybir.ActivationFunctionType.Exp,
                                    accum_out=sm[:])
                                sums[hf] = sm
                            rec = p2sb.tile([128, 1], F32, tag="rec")
                            if nhalf == 2:
                                tot = p2sb.tile([128, 1], F32, tag="tot")
                                nc.vector.tensor_add(tot[:], sums[0][:], sums[1][:])
                                nc.vector.reciprocal(rec[:], tot[:])
                            else:
                                nc.vector.reciprocal(rec[:], sums[0][:])
                            diag = p2sb.tile([128, 128], BF16, tag="diag")
                            nc.vector.tensor_scalar_mul(diag[:], ident_bf[:], rec[:])
                            # zero the causally-masked (never-written) segments
                            njt = 4 * (ir + 1)
                            for jt_z in range(it + 1, njt):
                                nc.vector.memset(
                                    pt_sb[:, jt_z, itl * 128:(itl + 1) * 128], 0.0)
                            # scaled transpose: P_tile.T @ diag(rec), bf16
                            for jg in range((it + 1 + 3) // 4):
                                q_n = min(4, it + 1 - jg * 4)
                                ps = ptps.tile([128, 512], F32, tag="ptp")
                                for q in range(q_n):
                                    jt = jg * 4 + q
                                    nc.tensor.matmul(
                                        ps[:, q * 128:(q + 1) * 128],
                                        p_sb[:, jt * 128:(jt + 1) * 128],
                                        diag[:],
                                        start=True, stop=True,
                                    )
                                nc.any.tensor_copy(
                                    pt_sb[:, jg * 4:jg * 4 + q_n, itl * 128:(itl + 1) * 128],
                                    ps[:, :q_n * 128].rearrange("p (q f) -> p q f", f=128))
                        # PV for this i-range (bf16)
                        pv = pvps.tile([128, 512], F32, tag="pv")
                        njt = 4 * (ir + 1)
                        for jt in range(njt):
                            nc.tensor.matmul(
                                pv[:],
                                VNB[i][:, jt, :],
                                pt_sb[:, jt, :],
                                start=(jt == 0), stop=(jt == njt - 1),
                            )
                        nc.any.tensor_copy(ATT[i][:, ir * 512:(ir + 1) * 512], pv[:])
                    if i == 2:
                        stage_a2a(0)   # head-slot 0 done -> overlap its A2A
                stage_a2a(1)

                # K natural-layout output (post-rope), overlaps with A2A #1
                for i in range(INST):
                    for jg in range(ITILES // 4):
                        ps = ptps.tile([128, 512], F32, tag="ptp")
                        for q in range(4):
                            jt = jg * 4 + q
                            nc.tensor.matmul(
                                ps[:, q * 128:(q + 1) * 128],
                                KT[i][:, jt * 128:(jt + 1) * 128],
                                ident_bf[:],
                                start=True, stop=True,
                            )
                        kb = p2sb.tile([128, 4, DK], F32, tag="kn")
                        # src col = blk*64 + t  ->  dst col = 2*t + blk
                        src = ps[:].rearrange("p (q blk t) -> p q blk t", q=4, blk=2)
                        dst = kb[:].rearrange("p q (h two) -> p q two h", two=2)
                        nc.any.tensor_copy(dst, src)
                        nc.sync.dma_start(
                            k_d[i].rearrange("(t p) f -> p t f", p=128)[:, jg * 4:jg * 4 + 4, :],
                            kb[:])

        # ---------------- phase 3: out_proj on my 512 rows ----------------
        # att_f layout: kt' = j*8+g  <->  head (2g+j); woT rows host-reordered.
        with tc.tile_pool(name="p3sb", bufs=1) as p3sb, \
             tc.tile_pool(name="p3w", bufs=2) as p3w, \
             tc.tile_pool(name="p3y", bufs=2) as p3y, \
             tc.tile_pool(name="yps", bufs=2, space="PSUM") as yps:
            att_f = p3sb.tile([128, NT, 512], BF16, tag="attf")
            for j in range(HPC):
                nc.sync.dma_start(
                    att_f[:, j * NC:(j + 1) * NC, :],
                    a2a_out[j].rearrange("g p n -> p g n"))
            for oh in range(2):
                wo_sb = p3w.tile([128, NT, 1024], BF16, tag="wo")
                nc.sync.dma_start(
                    wo_sb[:],
                    woT_d.rearrange("(t p) o -> p t o", p=128)[:, :, oh * 1024:(oh + 1) * 1024])
                for ntl in range(4):
                    y_ps = yps.tile([128, 1024], F32, tag="y")
                    for kt in range(NT):
                        for half in range(2):
                            nc.tensor.matmul(
                                y_ps[:, half * 512:(half + 1) * 512],
                                att_f[:, kt, ntl * 128:(ntl + 1) * 128],
                                wo_sb[:, kt, half * 512:(half + 1) * 512],
                                start=(kt == 0), stop=(kt == NT - 1),
                            )
                    yh = p3y.tile([128, 1024], F32, tag="yh")
                    nc.any.tensor_copy(yh[:], y_ps[:])
                    nc.sync.dma_start(
                        y_d[ntl * 128:(ntl + 1) * 128, oh * 1024:(oh + 1) * 1024],
                        yh[:])
    nc.finalize()
    return nc


_CACHE = {}


def _get_nc():
    if "nc" not in _CACHE:
        _CACHE["nc"] = build_kernel()
    return _CACHE["nc"]


def _rope_cache_np():
    half = DK // 2
    inv_freq = 1.0 / (10000.0 ** (np.arange(half, dtype=np.float32) / half))
    ang = np.arange(N, dtype=np.float32)[:, None] * inv_freq[None, :]
    return np.cos(ang).astype(np.float32), np.sin(ang).astype(np.float32)


def _make_in_maps(x, wq, wk, wv, wo):
    bf = ml_dtypes.bfloat16
    xT = np.ascontiguousarray(x.reshape(BN, D).T.astype(bf))      # [D, BN] bf16
    # wo rows reordered: block j*8+g holds head (2g+j)
    woT = wo.T.reshape(H, DK, D)                                   # head-major rows
    order = [2 * g + j for j in range(HPC) for g in range(NC)]
    woT = np.ascontiguousarray(woT[order].reshape(D, D).astype(bf))
    cos, sin = _rope_cache_np()
    # full-partition rope tables for the evens/odds-block permuted layout:
    # rows 0-63 (x1 block): cos, -sin ; rows 64-127 (x2 block): cos, +sin
    cosT = np.ascontiguousarray(np.concatenate([cos.T, cos.T], axis=0).astype(bf))
    sinT = np.ascontiguousarray(np.concatenate([-sin.T, sin.T], axis=0).astype(bf))

    # rope permutation within each head: evens then odds
    perm = np.concatenate([np.arange(0, DK, 2), np.arange(1, DK, 2)])
    invs = np.float32(1.0 / np.sqrt(np.float32(DK)))

    in_maps = []
    for c in range(NC):
        rows = slice(c * HPC * DK, (c + 1) * HPC * DK)
        wq_c = wq[rows].reshape(HPC, DK, D)[:, perm, :].reshape(HPC * DK, D) * invs
        wk_c = wk[rows].reshape(HPC, DK, D)[:, perm, :].reshape(HPC * DK, D)
        wv_c = wv[rows]
        in_maps.append({
            "xT": xT,
            "wqT": np.ascontiguousarray(wq_c.T.astype(bf)),
            "wkT": np.ascontiguousarray(wk_c.T.astype(bf)),
            "wvT": np.ascontiguousarray(wv_c.T.astype(bf)),
            "woT": woT,
            "cosT": cosT,
            "sinT": sinT,
        })
    return in_maps


def _assemble(results):
    out = np.empty((BN, D), dtype=np.float32)
    K = np.empty((B, H, N, DK), dtype=np.float32)
    V = np.empty((B, H, N, DK), dtype=np.float32)
    for c in range(NC):
        out[c * ROWS_PER_CORE:(c + 1) * ROWS_PER_CORE] = results[c]["y_part"]
        ko = results[c]["k_out"]
        vo = results[c]["v_out"]
        for b in range(B):
            for hh in range(HPC):
                K[b, HPC * c + hh] = ko[b * HPC + hh]
                V[b, HPC * c + hh] = vo[b * HPC + hh]
    return out.reshape(B, N, D), K, V


def kernel(x, wq, wk, wv, wo):
    nc = _get_nc()
    x = np.asarray(x, dtype=np.float32)
    wq = np.asarray(wq, dtype=np.float32)
    wk = np.asarray(wk, dtype=np.float32)
    wv = np.asarray(wv, dtype=np.float32)
    wo = np.asarray(wo, dtype=np.float32)
    in_maps = _make_in_maps(x, wq, wk, wv, wo)
    res = run_bass_kernel_spmd(nc, in_maps, core_ids=list(range(NC)))
    return _assemble(res.results)
